# revision 45
# baseline (speedup 1.0000x reference)
"""Trainium2 Bass kernel for the DifferentiableStarPlanner problem.

Key algorithmic facts (derived from the reference semantics):
  * The returned tensor is only ``g``; the open/close/maxpool frontier
    bookkeeping never feeds back into ``g`` (dead code for the output).
  * ``g`` starts at INF everywhere except the start seeds (value 0) and the
    3x3 min-plus stencil propagates information exactly one cell per sweep,
    so after ``n`` sweeps only cells within Chebyshev distance ``n`` of a
    seed can differ from INF.
  * Therefore the device only has to iterate the stencil on a small box
    around the seeds; everything else is INF background.

Device strategy (SPMD on 8 NeuronCores, identical program+data — the box is
small enough that any domain decomposition would cost more in halo than it
saves, so the cores run the box redundantly and core 0's result is used):
  * Box of 2n x 2n cells (128 x 128 for n=64): rows/cols [s-n, s+n-1].
    Rows live on the 128 SBUF partitions, columns on the free dim with one
    INF pad column on each side.
  * All arithmetic is fp16 scaled by 1/4 (reals reach ~20*OB ~ 2e5, so
    values/4 stay under a 60000 sentinel that maps back to 1e7; fp16's
    11-bit mantissa keeps the per-sweep rounding well inside the 2e-2 rel
    tolerance).
  * Per sweep the PE builds all nine candidate planes in PSUM and the DVE
    runs a SINGLE windowed 9-slot tensor_reduce(min) -> new state:
      - a "preload" matmul per PSUM bank (identity weights @ c_map windows)
        runs during the previous sweep's reduce and parks cm_s in each
        plane; start=True exploits the bank-granular lazy-zero semantics;
      - one shift matmul per dy group ([w_dn | I | w_up] fp16 weights, 0/1
        exact) with a multi-window rhs AP [[1,3],[1,w]] accumulates the
        dx in {-1,0,1} shifted state on top of all three of its bank's
        planes in one instruction;
      - the center plane's c_map slot is zero, so that plane IS the old
        state: the reduce includes the reference's outer min(g, .) and
        dominates the stay channel for free.
    Plane slots are packed 3 per 2 KiB PSUM bank so no matmul output ever
    crosses a bank boundary; the reduce reads the 3x3 slot lattice with a
    [[1,w],[512,3],[130,3]] AP reduced over XY.
  * Zeroed shift rows (gD row 0 / gU row 127) and rows outside a short box
    are neutralized by +INF* biases baked into the c_map rows.
  * Active-window scaling: after sweep t only cells within Chebyshev radius
    t+1 of a seed differ from INF*, and cells outside hold INF* in BOTH
    ping-pong state buffers, so every op runs on a window of min(2t+3, B).
  * The c_map (local path costs) is static across sweeps and precomputed on
    the host for the box only (including the reference's obstacle-slice
    quirk for the (0,-1) channel).
  * After the device loop, cells at Chebyshev radius exactly n (one row and
    one column strip just outside the device box) are first reached on the
    very last sweep; they are stitched on the host with one numpy stencil
    step from g_{n-1} (also output by the device).
  * Anything the fast path cannot prove exact (non-binary start_map, seeds
    near the grid edge, multi-seed boxes over 128, n < 2) falls back to an
    exact vectorized numpy port of the reference.
"""

import sys

import numpy as np

if "/opt/trn_rl_repo" not in sys.path and not any(
    p.endswith("trn_rl_repo") for p in sys.path
):
    sys.path.insert(0, "/opt/trn_rl_repo")

INF = np.float32(1.0e7)
OB = np.float32(1.0e4)
EPS = np.float32(1e-12)

# Neighbor gather offsets in reference channel order c = x*3 + y, skipping
# the center channel (redundant: the outer min already includes g itself).
_CH8 = [
    (dy, dx) for dx in (-1, 0, 1) for dy in (-1, 0, 1) if not (dy == 0 and dx == 0)
]

# Obstacle-slice offset used by each channel's cost in the reference
# (channel (0,-1) uses the (-1,0) obstacle slice — quirk preserved from the
# original source, ported faithfully here).
_OBST_OFF = {
    (-1, -1): (-1, -1),
    (0, -1): (-1, 0),
    (1, -1): (1, -1),
    (-1, 0): (-1, 0),
    (1, 0): (1, 0),
    (-1, 1): (-1, 1),
    (0, 1): (0, 1),
    (1, 1): (1, 1),
}


def _cmap_box(obs, x, y, r0, c0, nr, ncol):
    """c_map channels for box rows [r0, r0+nr), cols [c0, c0+ncol).

    Requires a 1-cell margin inside the grid on all sides. Returns a dict
    {(dy, dx): [nr, ncol] float32}, matching the reference arithmetic
    bit-for-bit (fp32 adds/sqrt in the same order).
    """
    xs = x[r0 - 1 : r0 + nr + 1, c0 - 1 : c0 + ncol + 1].astype(np.float32)
    ys = y[r0 - 1 : r0 + nr + 1, c0 - 1 : c0 + ncol + 1].astype(np.float32)
    os_ = obs[r0 - 1 : r0 + nr + 1, c0 - 1 : c0 + ncol + 1].astype(np.float32)

    def sh(a, dy, dx):
        return a[1 + dy : 1 + dy + nr, 1 + dx : 1 + dx + ncol]

    left = (sh(xs, 0, 0) - sh(xs, 0, -1)) ** 2
    right = (sh(xs, 0, 0) - sh(xs, 0, 1)) ** 2
    up = (sh(ys, 0, 0) - sh(ys, -1, 0)) ** 2
    down = (sh(ys, 0, 0) - sh(ys, 1, 0)) ** 2
    oc = sh(os_, 0, 0)

    def obst(dy, dx):
        return OB * np.maximum(sh(os_, dy, dx), oc)

    grad = {
        (-1, -1): left + up,
        (0, -1): left,
        (1, -1): left + down,
        (-1, 0): up,
        (1, 0): down,
        (-1, 1): right + up,
        (0, 1): right,
        (1, 1): right + down,
    }
    cm = {}
    for off in _CH8:
        d = np.sqrt(grad[off] + EPS).astype(np.float32)
        cm[off] = (d + obst(*_OBST_OFF[off])).astype(np.float32)
    return cm


def _reference_numpy(obstacles, coords, start_map, num_steps):
    """Exact vectorized numpy port of the reference (fallback path)."""
    H, W = obstacles.shape[2], obstacles.shape[3]
    obs = obstacles[0, 0].astype(np.float32)
    y = coords[0, 0].astype(np.float32)
    x = coords[0, 1].astype(np.float32)
    start = start_map[0, 0].astype(np.float32)

    xp = np.pad(x, ((0, 0), (1, 1)), mode="edge")
    yp = np.pad(y, ((1, 1), (0, 0)), mode="edge")
    left = (xp[:, 1:-1] - xp[:, :-2]) ** 2
    right = (xp[:, 1:-1] - xp[:, 2:]) ** 2
    up = (yp[1:-1, :] - yp[:-2, :]) ** 2
    down = (yp[1:-1, :] - yp[2:, :]) ** 2
    op = np.pad(obs, ((1, 1), (1, 1)), mode="edge")

    def nb(dy, dx, a=op):
        return a[1 + dy : 1 + dy + H, 1 + dx : 1 + dx + W]

    def obst(dy, dx):
        return OB * np.maximum(nb(dy, dx), obs)

    grad = {
        (-1, -1): left + up,
        (0, -1): left,
        (1, -1): left + down,
        (-1, 0): up,
        (1, 0): down,
        (-1, 1): right + up,
        (0, 1): right,
        (1, 1): right + down,
    }
    cmaps = {
        off: (np.sqrt(grad[off] + EPS) + obst(*_OBST_OFF[off])).astype(np.float32)
        for off in _CH8
    }
    c_center = (OB * obs).astype(np.float32)

    g = np.clip(INF * (np.float32(1.0) - start), 0.0, INF).astype(np.float32)
    for _ in range(int(num_steps)):
        gp = np.pad(g, ((1, 1), (1, 1)), mode="edge")
        acc = (g + c_center).astype(np.float32)
        for off in _CH8:
            acc = np.minimum(acc, nb(*off, a=gp) + cmaps[off])
        g = np.minimum(g, acc)
    return g


# Cache of built device programs keyed by (n, B) so repeated kernel() calls
# do not rebuild the Bass module.
_DEVICE_CACHE = {}


def _win_ap(tile_ap, dims, offset_elems=0):
    """Return a copy of tile_ap with custom free dims [[step, count], ...]
    (keeps the partition dim), allowing overlapping/windowed reads."""
    import concourse.mybir as mybir

    c = tile_ap.copy()
    part = list(c.ap[0])
    c.ap = mybir.VecI64Pair([part] + [list(d) for d in dims])
    if offset_elems:
        c.offset = c.offset + offset_elems
    return c


def _build_device(n, B, sc_lo, sc_hi):
    import concourse.mybir as mybir
    import concourse.tile as tile
    from concourse import bacc

    F = B + 2
    f32 = mybir.dt.float32
    f16 = mybir.dt.float16
    nc = bacc.Bacc("TRN2", target_bir_lowering=False, debug=False)

    d_g0 = nc.dram_tensor("g0", [128, F], f16, kind="ExternalInput")
    d_cm = nc.dram_tensor("cmap", [128, 9 * B], f16, kind="ExternalInput")
    d_w3 = nc.dram_tensor("w3", [128, 384], f16, kind="ExternalInput")
    # Only g_{n-1} leaves the device: the host replays the final sweep (and
    # the radius-n strips) itself, which is cheaper than one more full-width
    # device sweep plus a second output DMA.
    d_gn1 = nc.dram_tensor("g_nm1", [128, B], f16, kind="ExternalOutput")

    def window(t):
        c_lo = max(0, sc_lo - (t + 1))
        c_hi = min(B - 1, sc_hi + (t + 1))
        return c_lo, c_hi - c_lo + 1

    with tile.TileContext(nc) as tc:
        with (
            tc.tile_pool(name="state", bufs=1) as state,
            tc.tile_pool(name="psum", bufs=1, space="PSUM") as psum,
        ):
            # State ping-pong tiles (fp16, one F-wide row each: col 0 / F-1
            # are the INF* pad columns).
            ST = [
                state.tile([128, F], f16, tag="ST0", name="ST0"),
                state.tile([128, F], f16, tag="ST1", name="ST1"),
            ]
            # Static per-channel cost maps, 9 B-wide slots (slot 4 = zeros so
            # the center plane IS the old state and folds the outer min).
            cmS = state.tile([128, 9 * B], f16, tag="cmS")
            cm = state.tile([128, 9 * B], f16, tag="cm")
            # Weights [w_dn | I | w_up]: w_dn shifts rows down (gD[p]=g[p-1]),
            # I copies, w_up shifts rows up (gU[p]=g[p+1]).
            w3s = state.tile([128, 384], f16, tag="w3s")
            w3 = state.tile([128, 384], f16, tag="w3")
            # Candidate planes: nine fp32 slots per sweep parity. Plane s
            # holds (shifted state + cm_s) for channel s, built by a cm
            # preload matmul (identity weights, runs during the previous
            # sweep's reduce) plus an accumulating shift matmul.
            #
            # PSUM "start" marks the whole 2 KiB zero region lazily-zero, so
            # slots are packed 3 per bank (3*F floats <= 512) and only the
            # first preload in each bank carries start=True; everything else
            # accumulates (writes to still-pending bytes overwrite, which is
            # exactly what the non-preloaded center slot needs).
            BANK = 512  # fp32 elems per 2 KiB PSUM bank
            assert 3 * F <= BANK

            P = [
                psum.tile([128, 3 * BANK], f32, tag="PA", name="PA"),
                psum.tile([128, 3 * BANK], f32, tag="PB", name="PB"),
            ]

            # Issue the three input DMAs from three different engines so they
            # ride parallel DGE queues (same-engine DMAs serialize FIFO and
            # their init latencies stack up to several us).
            nc.sync.dma_start(cmS[:], d_cm[:])
            nc.scalar.dma_start(w3s[:], d_w3[:])
            nc.gpsimd.dma_start(ST[1][:], d_g0[:])
            # The LdWeights ISA slot carries a single sync wait, so every
            # matmul operand must be last-written by ONE engine: funnel the
            # DMA-loaded matmul operands through DVE copies.
            nc.vector.tensor_copy(ST[0][:], ST[1][:])
            nc.vector.tensor_copy(w3[:], w3s[:])
            nc.vector.tensor_copy(cm[:], cmS[:])

            mn = mybir.AluOpType.min
            I128 = w3[:, 128:256]

            def preload(buf, t):
                # cm -> psum planes for sweep t, one matmul per PSUM bank
                # covering its three slots ([[F,3],[1,w]] strided output,
                # matching [[B,3],[1,w]] cm windows; the center slot's cm is
                # zero). start=True marks the bank pending-zero (lazy zero);
                # the shift matmuls then accumulate on top.
                c_lo, w = window(t)
                for gi in range(3):
                    out = _win_ap(buf[:], [[F, 3], [1, w]], gi * BANK + c_lo)
                    rhs = _win_ap(cm[:], [[B, 3], [1, w]], gi * 3 * B + c_lo)
                    nc.tensor.matmul(
                        out,
                        I128,
                        rhs,
                        start=True,
                        stop=False,
                        skip_group_check=True,
                    )

            preload(P[0], 0)

            n_dev = n - 1  # the host replays the final sweep
            for t in range(n_dev):
                src = ST[t % 2]
                dst = ST[(t + 1) % 2]
                buf = P[t % 2]
                nbuf = P[(t + 1) % 2]
                c_lo, w = window(t)
                # Shift matmuls: plane s += state[row+dy] read at col+dx
                # (dx via the rhs column offset; fp16 weights 0/1 are exact).
                # gD row 0 / gU row 127 contract to 0 and are neutralized by
                # the host's +INF* bias baked into those cm rows.
                # One shift matmul per dy-group: the rhs multi-window AP
                # [[1,3],[1,w]] reads the state at dx in {-1,0,1} per slot,
                # accumulating onto the preloaded cm planes of bank gi.
                # Fewer, larger matmuls also shrink the post-PE pipeline
                # drain (min_engine_delay is max(busy, 173ns)).
                for gi in range(3):  # dy = gi - 1
                    lhsT = w3[:, gi * 128 : (gi + 1) * 128]
                    out = _win_ap(buf[:], [[F, 3], [1, w]], gi * BANK + c_lo)
                    rhs = _win_ap(src[:], [[1, 3], [1, w]], c_lo)
                    nc.tensor.matmul(
                        out,
                        lhsT,
                        rhs,
                        start=False,
                        stop=True,
                        skip_group_check=True,
                    )
                # cm preload for the next sweep runs on PE while DVE reduces.
                if t + 1 < n_dev:
                    preload(nbuf, t + 1)

                # The whole sweep's min (9 candidate planes; plane 4 is the
                # old state, folding the reference's outer min) in a single
                # windowed reduce over the bank-packed slot lattice.
                red_in = _win_ap(buf[:], [[1, w], [BANK, 3], [F, 3]], c_lo)
                nc.vector.tensor_reduce(
                    dst[:, 1 + c_lo : 1 + c_lo + w],
                    red_in,
                    axis=mybir.AxisListType.XY,
                    op=mn,
                )
                if t == n_dev - 1:
                    # Split the output across two engines' DGE queues so the
                    # two row-halves transfer in parallel.
                    nc.sync.dma_start(d_gn1[0:64, :], dst[0:64, 1 : 1 + B])
                    nc.scalar.dma_start(d_gn1[64:128, :], dst[64:128, 1 : 1 + B])

    nc.compile()
    return nc


def _make_runner(nc, n_cores=8):
    """Build a cached jitted runner for the Bass module (the stock
    run_bass_kernel_spmd path re-traces jax on every call; this one traces
    once and reuses the executable)."""
    import jax
    import numpy as _np
    from concourse import bass2jax
    from concourse.bass2jax import _bass_exec_p, partition_id_tensor
    from jax.experimental.shard_map import shard_map
    from jax.sharding import Mesh, PartitionSpec

    import concourse.mybir as mybir

    bass2jax.install_neuronx_cc_hook()

    partition_name = nc.partition_id_tensor.name if nc.partition_id_tensor else None
    in_names, out_names, out_avals, zero_shapes = [], [], [], []
    for alloc in nc.m.functions[0].allocations:
        if not isinstance(alloc, mybir.MemoryLocationSet):
            continue
        name = alloc.memorylocations[0].name
        if alloc.kind == "ExternalInput":
            if name != partition_name:
                in_names.append(name)
        elif alloc.kind == "ExternalOutput":
            out_names.append(name)
            shape = tuple(alloc.tensor_shape)
            dtype = mybir.dt.np(alloc.dtype)
            out_avals.append(jax.core.ShapedArray(shape, dtype))
            zero_shapes.append((shape, dtype))
    n_params = len(in_names)
    n_outs = len(out_avals)
    all_names = list(in_names) + list(out_names)
    if partition_name is not None:
        all_names.append(partition_name)
    donate = tuple(range(n_params, n_params + n_outs))

    def _body(*args):
        operands = list(args)
        if partition_name is not None:
            operands.append(partition_id_tensor())
        outs = _bass_exec_p.bind(
            *operands,
            out_avals=tuple(out_avals),
            in_names=tuple(all_names),
            out_names=tuple(out_names),
            lowering_input_output_aliases=(),
            sim_require_finite=True,
            sim_require_nnan=True,
            nc=nc,
        )
        return tuple(outs)

    n_cores = min(n_cores, len(jax.devices()))
    devices = jax.devices()[:n_cores]
    mesh = Mesh(_np.asarray(devices), ("core",))
    sharded = jax.jit(
        shard_map(
            _body,
            mesh=mesh,
            in_specs=(PartitionSpec("core"),) * (n_params + n_outs),
            out_specs=(PartitionSpec("core"),) * n_outs,
            check_rep=False,
        ),
        donate_argnums=donate,
        keep_unused=True,
    )

    def run(in_map):
        concat_in = [
            _np.concatenate([_np.asarray(in_map[name])] * n_cores, axis=0)
            for name in in_names
        ]
        concat_zeros = [
            _np.zeros((n_cores * s[0], *s[1:]), d) for (s, d) in zero_shapes
        ]
        out_arrs = sharded(*concat_in, *concat_zeros)
        return {
            name: _np.asarray(out_arrs[i])[: out_avals[i].shape[0]]
            for i, name in enumerate(out_names)
        }

    return run


def _run_device(n, B, sc_lo, sc_hi, g0p, cmaps, w3):
    key = (n, B, sc_lo, sc_hi)
    if key not in _DEVICE_CACHE:
        nc = _build_device(n, B, sc_lo, sc_hi)
        _DEVICE_CACHE[key] = _make_runner(nc)
    run = _DEVICE_CACHE[key]
    out0 = run({"g0": g0p, "cmap": cmaps, "w3": w3})
    return out0["g_nm1"]


def kernel(obstacles, coords, start_map, goal_map, num_steps):
    obstacles = np.asarray(obstacles)
    coords = np.asarray(coords)
    start_map = np.asarray(start_map)
    n = int(np.asarray(num_steps))
    H, W = obstacles.shape[2], obstacles.shape[3]

    obs = obstacles[0, 0].astype(np.float32)
    y = coords[0, 0].astype(np.float32)
    x = coords[0, 1].astype(np.float32)
    start = start_map[0, 0].astype(np.float32)

    g_init = np.clip(INF * (np.float32(1.0) - start), 0.0, INF).astype(np.float32)

    seeds = np.argwhere(start > 0.5)
    if len(seeds) == 0 or n <= 0:
        return g_init[None, None].astype(np.float32)

    rmin, cmin = seeds.min(axis=0)
    rmax, cmax = seeds.max(axis=0)
    # Device box M: rows [r0, r0+Br), cols [c0, c0+Bc); cells at Chebyshev
    # distance exactly n from the seed set form one row and one column strip
    # just outside M (stitched on the host from g_{n-1}).
    r0 = int(rmin) - n
    c0 = int(cmin) - n
    Br = int(rmax - rmin) + 2 * n
    Bc = int(cmax - cmin) + 2 * n

    # The fast path relies on non-seed cells starting at exactly 1e7
    # (binary start_map) so the outside-box region provably never changes.
    start_binary = bool(np.all((start == 0.0) | (start == 1.0)))
    usable = (
        start_binary
        and Br == Bc
        and Br <= 128
        and n >= 2
        and r0 >= 1
        and c0 >= 1
        and r0 + Br + 1 <= H - 1
        and c0 + Bc + 1 <= W - 1
    )
    if not usable:
        return _reference_numpy(obstacles, coords, start_map, n)[None, None]

    B = Br
    # c_map over the big box (rows r0..r0+B inclusive) — the extra row/col
    # feeds the host stitch of the radius-n strip.
    cm_big = _cmap_box(obs, x, y, r0, c0, B + 1, B + 1)
    # On-device arithmetic is fp16 with all values scaled by 1/4 so the full
    # value range (reals up to ~20*OB + geometry, INF* sentinel above that)
    # fits fp16's finite range:
    #   real candidates  <= (20*OB + geo + OB + sqrt2)/4 ~ 52500 < THRESH
    #   sentinel INF*     = 60000 (exact in fp16)
    #   biased/INF cands <= 60000 + 2501 < 65504 (finite)
    # Host maps device values v -> 1e7 if v >= THRESH else 4*v.
    SCALE = np.float32(0.25)
    S_INF = np.float32(60000.0)
    # Device c_map slots s = (dy+1)*3 + (dx+1); slot 4 (center) is zero so
    # the center plane is exactly the old state (folds the outer min and
    # dominates the reference's stay channel).
    slots = [
        (-1, -1), (-1, 0), (-1, 1),
        (0, -1), (0, 0), (0, 1),
        (1, -1), (1, 0), (1, 1),
    ]
    cm_dev = np.zeros((9, 128, B), np.float32)
    if B < 128:
        # Rows >= B: all neighbor candidates biased +INF* so the center-plane
        # (state) min keeps those rows exactly at the sentinel.
        for s in range(9):
            if s != 4:
                cm_dev[s, B:, :] = S_INF
    for s, (dy, dx) in enumerate(slots):
        if s == 4:
            continue
        cm_dev[s, :B, :] = cm_big[(dy, dx)][:B, :B] * SCALE
        # Neutralize the zeroed gD row 0 / gU row 127: bias those channels'
        # costs by +INF* so their candidates lose to every real value and to
        # the sentinel itself (INF-neighbor candidates in the reference).
        if dy == -1:
            cm_dev[s, 0, :] += S_INF
        if dy == 1 and B == 128:
            cm_dev[s, B - 1, :] += S_INF
    # Device layout: one contiguous [128, 9B] fp16 buffer (slot-major per
    # row) so the whole c_map loads in a single DMA.
    cm_dev = np.ascontiguousarray(
        cm_dev.transpose(1, 0, 2).reshape(128, 9 * B)
    ).astype(np.float16)

    F = B + 2
    g0p = np.full((128, F), S_INF, np.float16)
    blk = g_init[r0 : r0 + B, c0 : c0 + B]
    g0p[:B, 1 : 1 + B] = np.where(blk >= INF, S_INF, blk * SCALE).astype(np.float16)

    w_dn = np.eye(128, 128, k=1, dtype=np.float16)  # gD[p] = g[p-1], gD[0] = 0
    w_id = np.eye(128, 128, dtype=np.float16)
    w_up = np.eye(128, 128, k=-1, dtype=np.float16)  # gU[p] = g[p+1], gU[127] = 0
    w3 = np.ascontiguousarray(np.concatenate([w_dn, w_id, w_up], axis=1))

    sc_lo = int(cmin) - c0
    sc_hi = int(cmax) - c0
    g_nm1 = _run_device(n, B, sc_lo, sc_hi, g0p, cm_dev, w3)
    THRESH = np.float32(57344.0)

    def unmap(v):
        v = np.asarray(v)[:B].astype(np.float32)
        return np.where(v >= THRESH, INF, v * np.float32(4.0))

    g_nm1 = unmap(g_nm1)

    # Host replays the reference's final sweep over the box (INF outside the
    # box is the true value of those cells at sweep n-1, so a constant-INF
    # pad is exact; the center/stay channel is dominated by the outer min).
    Gp = np.full((B + 2, B + 2), INF, np.float32)
    Gp[1 : 1 + B, 1 : 1 + B] = g_nm1
    acc = np.full((B, B), INF, np.float32)
    for off in _CH8:
        dy, dx = off
        acc = np.minimum(
            acc, Gp[1 + dy : 1 + dy + B, 1 + dx : 1 + dx + B] + cm_big[off][:B, :B]
        )
    g_n = np.minimum(g_nm1, acc).astype(np.float32)

    out = np.full((H, W), INF, np.float32)
    out[r0 : r0 + B, c0 : c0 + B] = g_n

    # Host stitch: one stencil step for the radius-n strip (row r0+B and col
    # c0+B of the big box), fed by g_{n-1} (INF outside M).
    G = np.full((B + 3, B + 3), INF, np.float32)  # index shift +1
    G[1 : 1 + B, 1 : 1 + B] = g_nm1

    # Row strip: cells (r0+B, c0+j) for j in 0..B  -> big-box local (B, 0..B)
    # The center channel and the min with the previous value are both INF on
    # the strip (first reached this very sweep), so only the 8 neighbor
    # channels matter.
    row_acc = np.full(B + 1, INF, np.float32)
    col_acc = np.full(B + 1, INF, np.float32)
    jj = np.arange(B + 1)
    for off in _CH8:
        dy, dx = off
        cand_r = G[1 + B + dy, 1 + jj + dx] + cm_big[off][B, :]
        row_acc = np.minimum(row_acc, cand_r.astype(np.float32))
        cand_c = G[1 + jj + dy, 1 + B + dx] + cm_big[off][:, B]
        col_acc = np.minimum(col_acc, cand_c.astype(np.float32))
    out[r0 + B, c0 : c0 + B + 1] = row_acc
    out[r0 : r0 + B + 1, c0 + B] = col_acc

    return out[None, None].astype(np.float32)

